# revision 1
# baseline (speedup 1.0000x reference)
"""Trainium2 Bass kernel for the DVS-SNN problem.

Model (per reference):
    for t in 0..T-1:
        i1 = x_t @ w1.T + spk @ w_rec.T
        v1 = v1 + i1 - LEAK ; spk = (v1 >= 1) ; v1 -= spk
        v2 = max(v2 + spk @ w2.T - OUTPUT_LEAK, 0) ; out_sum += v2
    return out_sum / T

Strategy: data-parallel over batch (64 = 8 cores x 8). Per core, one
interleaved program:
  Phase A (time-parallel): nxp[h, 32t+8j+b] = -(S*(xp - LEAK)).
      Main pass: fp16 xh @ (-S*w1h). Correction pass: ONE DoubleRow fp8
      matmul per (k,j) computes w1h*xl + w1l*xh (the two fp16-residual
      cross terms) at e4m3 precision -- enough to stay below the spike
      dynamics' chaos cliff (sim: rel err 6.8e-3 vs 5.4e-2 saturated).
      Combine on ScalarE (scale+bias) + one DVE add into nxp. The A work
      of tile n is EMITTED across the scan steps of tile n-1 so the Tile
      scheduler packs it into the scan's PE gaps. x is streamed in three
      windows per chunk (few, large DMAs, spread over 3 HWDGE queues).
  Phase B (sequential scan): state nU = -(S*v1 + XP_next), tiles
      [128, 32]. w_rec fp16 (scaled S=64) stationaries, fp8 spike moving
      (mixed-dtype matmul). Critical path per step: ONE fused DVE op
      spk = (psB - TH_S >= nU), then the 16 LDW+MM burst. Three DVE
      update ops run in the burst's shadow.
  Phase C (per column tile, interleaved): i2 = spk @ w2.T with fp16 w2
      stationary x fp8 spikes; (i2-OL)/T on ScalarE; final v2 relu-scan +
      reduce at the end.
"""

import os
import numpy as np

B, T, C, H, O = 64, 256, 2048, 512, 11
THRESHOLD = 1.0
LEAK = 0.003
OUTPUT_LEAK = LEAK * 0.5

NCORES = 8
BL = B // NCORES          # batch per core = 8
BT = T * BL               # 2048 moving columns per core
P = 128
KC = C // P               # 16 contraction chunks for phase A
KH = H // P               # 4 H chunks
NTILE = 512               # max phase A psum tile (64 timesteps x 8 batch)
XWINS = [(0, 20), (20, 36), (36, 128), (128, 256)]  # x windows (timesteps)
XWMAX = 128               # max window width, timesteps

SCALE = 64.0              # v1 dynamics scale (wrec fp16 subnormal safety)
# fp8 correction-factor scalings (all powers of two; products land at
# -S * 2^CSH * (w1h*xl + w1l*xh)).
SH_W, SH_WL, SH_X, SH_XL = 5, 16, 3, 14
CSH = float(2 ** (SH_W + SH_XL))  # = 2^(SH_WL+SH_X) = 2^19


def _sched(t_steps):
    """Column tiles: smooth ramp (producer window >= ~0.7x consumer),
    aligned to the x-stream window boundaries."""
    assert t_steps == 256
    bounds = [0, 20, 36, 60, 92, 128, 172, 216, 256]
    return list(zip(bounds[:-1], bounds[1:]))


def build_nc(t_steps=T):
    """Build the Bass program (same program for all 8 cores)."""
    from contextlib import ExitStack

    import concourse.tile as tile
    from concourse import bacc, mybir

    f32 = mybir.dt.float32
    f16 = mybir.dt.float16
    f8 = mybir.dt.float8e4
    alu = mybir.AluOpType
    ACT = mybir.ActivationFunctionType
    DR = mybir.MatmulPerfMode.DoubleRow

    nc = bacc.Bacc("TRN2", target_bir_lowering=False, debug=False,
                   num_devices=NCORES)

    # ---- DRAM I/O ----
    xh_d = nc.dram_tensor("xh", [C, BT], f16, kind="ExternalInput")
    xdr_d = nc.dram_tensor("xdr", [C, 2, BT], f8, kind="ExternalInput")
    w1m_d = nc.dram_tensor("w1m", [C, H], f16, kind="ExternalInput")
    wdr_d = nc.dram_tensor("wdr", [C, 2, H], f8, kind="ExternalInput")
    wrt_d = nc.dram_tensor("wrt", [H, H], f16, kind="ExternalInput")
    w2t_d = nc.dram_tensor("w2t", [H, O], f16, kind="ExternalInput")
    out_d = nc.dram_tensor("out", [O, BL], f32, kind="ExternalOutput")

    TH_S = SCALE * THRESHOLD
    sched = _sched(t_steps)

    with tile.TileContext(nc) as tc, ExitStack() as ctx:
        perm = ctx.enter_context(tc.tile_pool(name="perm", bufs=1))

        def ptile(shape, dt_, tag):
            return perm.tile(shape, dt_, tag=tag, name=tag)

        nxp = ptile([P, 32 * t_steps], f32, "nxp")          # -(S*(xp-LEAK))
        spk8 = ptile([P, 32 * t_steps], f8, "spk8")         # {0,1} spikes
        nU = ptile([P, 32], f32, "nU")                      # scan state
        # phase C deltas, padded: per batch, T deltas then a -inf sentinel
        # column so ONE flat relu-scan handles all batches back-to-back.
        d_pad = ptile([O, BL * (t_steps + 1)], f32, "d_pad")

        w1m_sb, wdr_sb = [], []
        wr_sb, w2_sb = [], []
        xwin = {}                                            # (k) -> tiles

        xpool = ctx.enter_context(tc.tile_pool(name="xp", bufs=1))
        tmp_pool = ctx.enter_context(tc.tile_pool(name="tmpA", bufs=2))
        psA_pool = ctx.enter_context(tc.tile_pool(name="psA", bufs=1,
                                                  space="PSUM"))
        psB_pool = ctx.enter_context(tc.tile_pool(name="psB", bufs=2,
                                                  space="PSUM"))
        psV_pool = ctx.enter_context(tc.tile_pool(name="psV", bufs=2,
                                                  space="PSUM"))

        dma_engines = [nc.sync, nc.scalar, nc.gpsimd]

        def dma_xwin(w, k, eng=None):
            """Stream x window w for contraction chunk k on a HWDGE queue."""
            w0, w1 = XWINS[w]
            wl = (w1 - w0) * BL
            eng = eng if eng is not None else dma_engines[k % 3]
            xh_t = xpool.tile([P, wl], f16, tag=f"xh_{k}", name=f"xh_{k}",
                              padded_shape=[P, XWMAX * BL])
            xd_t = xpool.tile([P, 2, wl], f8, tag=f"xd_{k}", name=f"xd_{k}",
                              padded_shape=[P, 2, XWMAX * BL])
            csl = slice(k * P, (k + 1) * P)
            nsl = slice(w0 * BL, w1 * BL)
            eng.dma_start(out=xh_t[:], in_=xh_d.ap()[csl, nsl])
            eng.dma_start(out=xd_t[:], in_=xdr_d.ap()[csl, :, nsl])
            xwin[(w, k)] = (xh_t, xd_t)

        # ---------- Phase A work for one column tile, as a work list ----------
        def a_tile_work(s0, s1):
            ntile = (s1 - s0) * BL
            w = next(i for i, (a, b) in enumerate(XWINS) if a <= s0 < b)
            base = s0 - XWINS[w][0]
            psA = [None] * 2
            psC = [None] * 2
            items = []
            for jp in range(2):
                def alloc(jp=jp):
                    for jj in range(2):
                        psA[jj] = psA_pool.tile([P, ntile], f32,
                                                tag=f"psA{jj}", name="psA",
                                                padded_shape=[P, NTILE])
                        psC[jj] = psA_pool.tile([P, ntile], f32,
                                                tag=f"psC{jj}", name="psC",
                                                padded_shape=[P, NTILE])
                items.append(alloc)
                # narrow (ramp) tiles: DoubleRow's 256-col no-FWL LDWEIGHTS
                # dominates tiny moving widths -- use two regular fp8
                # matmuls (27ns FWL LDW) on the interleaved slices instead.
                use_dr = (s1 - s0) >= 32
                for k in range(KC):
                    for jj in range(2):
                        def mms(k=k, jp=jp, jj=jj):
                            xh_t, xd_t = xwin[(w, k)]
                            xsl = slice(base * BL, base * BL + ntile)
                            j = 2 * jp + jj
                            jsl = slice(j * P, (j + 1) * P)
                            nc.tensor.matmul(
                                psA[jj][:], w1m_sb[k][:][:, jsl],
                                xh_t[:, xsl],
                                start=(k == 0), stop=(k == KC - 1))
                            if use_dr:
                                nc.tensor.matmul(
                                    psC[jj][:], wdr_sb[k][:][:, :, jsl],
                                    xd_t[:, :, xsl],
                                    start=(k == 0), stop=(k == KC - 1),
                                    perf_mode=DR)
                            else:
                                nc.tensor.matmul(
                                    psC[jj][:], wdr_sb[k][:][:, 0, jsl],
                                    xd_t[:, 0, xsl],
                                    start=(k == 0), stop=False)
                                nc.tensor.matmul(
                                    psC[jj][:], wdr_sb[k][:][:, 1, jsl],
                                    xd_t[:, 1, xsl],
                                    start=False, stop=(k == KC - 1))
                        items.append(mms)
                for jj in range(2):
                    def combine(jj=jj, jp=jp):
                        j = 2 * jp + jj
                        tmp = tmp_pool.tile([P, ntile], f32, tag="tmpA",
                                            name="tmpA",
                                            padded_shape=[P, NTILE])
                        nc.scalar.activation(tmp[:], psC[jj][:], ACT.Copy,
                                             bias=SCALE * LEAK,
                                             scale=1.0 / CSH)
                        # the DVE add is split into <=128-col pieces so it
                        # never blocks the scan's critical compare for long
                        # in the strict-FIFO vector queue
                        dest = nxp[:].rearrange("p (t j b) -> p t j b",
                                                j=KH, b=BL)
                        pA = psA[jj][:].rearrange("p (t b) -> p t b", b=BL)
                        tA = tmp[:].rearrange("p (t b) -> p t b", b=BL)
                        nst = s1 - s0
                        for q0 in range(0, nst, 16):
                            q1 = min(q0 + 16, nst)
                            nc.vector.tensor_tensor(
                                dest[:, s0 + q0:s0 + q1, j, :],
                                pA[:, q0:q1, :], tA[:, q0:q1, :], alu.add)
                    items.append(combine)
            return items

        # ---------- Phase C work for one column tile (5 spread items) ------
        def c_tile_items(s0, s1):
            spk_r = spk8[:].rearrange("p (t c b) -> p t c b", c=KH, b=BL)
            hold = {}

            def mk_mm(k):
                def mm(k=k):
                    if k == 0:
                        hold["psV"] = psV_pool.tile(
                            [O, (s1 - s0) * BL], f32, tag="psV",
                            name="psV", padded_shape=[O, NTILE])
                    nc.tensor.matmul(hold["psV"][:], w2_sb[k][:],
                                     spk_r[:, s0:s1, k, :],
                                     start=(k == 0), stop=(k == KH - 1))
                return mm

            def fin():
                # d[o, b*(T+1) + t] = (i2 - OL)/T, written batch-major
                dest = d_pad[:].rearrange("o (b tp) -> o b tp",
                                          tp=t_steps + 1)[:, :, s0:s1]
                srcv = hold["psV"][:].rearrange("o (t b) -> o b t", b=BL)
                nc.scalar.activation(dest, srcv, ACT.Copy,
                                     bias=-OUTPUT_LEAK / float(T),
                                     scale=1.0 / float(T))
            return [mk_mm(k) for k in range(KH)] + [fin]

        # ---------- Weight + first-window DMAs ----------
        # Round-robin each chunk's four tensors across the three HWDGE
        # queues so the serial ~0.8us per-dma_start issue cost is split 3
        # ways and chunk k's data lands in chunk order.
        for k in range(KC):
            w1m_k = ptile([P, H], f16, f"w1m_{k}")
            wdr_k = ptile([P, 2, H], f8, f"wdr_{k}")
            dma_engines[k % 3].dma_start(
                out=w1m_k[:], in_=w1m_d.ap()[k * P:(k + 1) * P, :])
            dma_engines[(k + 1) % 3].dma_start(
                out=wdr_k[:], in_=wdr_d.ap()[k * P:(k + 1) * P, :, :])
            w1m_sb.append(w1m_k)
            wdr_sb.append(wdr_k)
            dma_xwin(0, k, eng=dma_engines[(k + 2) % 3])
        for k in range(KH):
            wr_k = ptile([P, H], f16, f"wr_{k}")
            nc.sync.dma_start(out=wr_k[:], in_=wrt_d.ap()[k * P:(k + 1) * P, :])
            wr_sb.append(wr_k)
            w2_k = ptile([P, O], f16, f"w2_{k}")
            nc.sync.dma_start(out=w2_k[:], in_=w2t_d.ap()[k * P:(k + 1) * P, :])
            w2_sb.append(w2_k)
        # phase C sentinels: a big negative delta between batch blocks
        # resets the relu accumulator to 0 at each batch boundary.
        nc.vector.memset(
            d_pad[:].rearrange("o (b tp) -> o b tp",
                               tp=t_steps + 1)[:, :, t_steps:t_steps + 1],
            -1e30)

        # ---------- Build per-step interleave schedule ----------
        # Tile n's A work is emitted across the scan steps of tile n-1
        # (disjoint windows: one tile's PSUM accumulation at a time);
        # tiles 0 and 1 are emitted up front.
        step_work = [[] for _ in range(t_steps + 6)]

        def spread(items, lo, hi):
            span = hi - lo
            for i, it in enumerate(items):
                step_work[lo + (i * span) // len(items)].append(it)

        pre_work = []
        for n, (s0, s1) in enumerate(sched):
            items = a_tile_work(s0, s1)
            if n == 0:
                pre_work.extend(items)
            else:
                spread(items, sched[n - 1][0], s0)
            for i, cit in enumerate(c_tile_items(s0, s1)):
                step_work[s1 + i].append(cit)
        # Later x windows: emit window w's DMAs just BEFORE the first
        # window-w tile's work items (same step, prepended). Slots are
        # released by the previous window's readers at runtime.
        for w in range(1, len(XWINS)):
            first_tile = next(n for n, (s0, _) in enumerate(sched)
                              if s0 >= XWINS[w][0])
            at = sched[first_tile - 1][0]
            for k in reversed(range(KC)):
                step_work[at].insert(0, lambda w=w, k=k: dma_xwin(w, k))

        for it in pre_work:
            it()

        # ---------- The scan ----------
        nc.vector.tensor_scalar(nU[:], nxp[:, 0:32], 0.0, None, alu.add)
        psB_prev = None
        for t in range(t_steps):
            for it in step_work[t]:
                it()
            sl = slice(32 * t, 32 * t + 32)
            if t == 0:
                nc.vector.tensor_scalar(spk8[:, sl], nU[:], -TH_S, None,
                                        alu.is_le)
            else:
                nc.vector.scalar_tensor_tensor(spk8[:, sl], psB_prev[:],
                                               -TH_S, nU[:], alu.add,
                                               alu.is_ge)
            if t + 1 < t_steps:
                # ---- PE burst: rec for step t+1 (fp16 x fp8 spikes) ----
                psB = psB_pool.tile([P, KH * BL], f32, tag="psB", name="psB")
                for j in range(KH):
                    for k in range(KH):
                        nc.tensor.matmul(
                            psB[:, BL * j:BL * (j + 1)],
                            wr_sb[k][:][:, j * P:(j + 1) * P],
                            spk8[:, 32 * t + BL * k:32 * t + BL * (k + 1)],
                            start=(k == 0), stop=(k == KH - 1))
                # ---- state update (off critical path) ----
                nc.vector.scalar_tensor_tensor(nU[:], spk8[:, sl], TH_S,
                                               nU[:], alu.mult, alu.add)
                if psB_prev is not None:
                    nc.vector.tensor_tensor(nU[:], nU[:], psB_prev[:],
                                            alu.subtract)
                nc.vector.tensor_add(nU[:], nU[:],
                                     nxp[:, 32 * (t + 1):32 * (t + 1) + 32])
                psB_prev = psB
        for tw in range(t_steps, t_steps + 6):
            for it in step_work[tw]:
                it()

        # ---------- Phase C tail ----------
        # One flat relu-scan over [O, BL*(T+1)]: the -1e30 sentinel columns
        # reset the accumulator at batch boundaries. Then a per-batch
        # reduce via a 3D view. No transpose DMAs needed.
        with tc.tile_pool(name="phC", bufs=1) as phC_pool:
            npad = BL * (t_steps + 1)
            zeros = phC_pool.tile([O, npad], f32, tag="zeros", name="zeros")
            v2a = phC_pool.tile([O, npad], f32, tag="v2a", name="v2a")
            osum = phC_pool.tile([O, BL], f32, tag="osum", name="osum")
            nc.vector.memset(zeros[:], 0.0)
            nc.vector.tensor_tensor_scan(v2a[:], d_pad[:], zeros[:], 0.0,
                                         alu.add, alu.max)
            v2a_r = v2a[:].rearrange("o (b tp) -> o b tp", tp=t_steps + 1)
            nc.vector.tensor_reduce(out=osum[:], in_=v2a_r,
                                    axis=mybir.AxisListType.X, op=alu.add)
            nc.sync.dma_start(out=out_d.ap()[:, :], in_=osum[:])

    nc.compile()
    return nc


def prep_inputs(x, w1, w_rec, w2):
    """Build per-core input maps. Host-side transposes/casts (not timed)."""
    from concourse import mybir
    f8np = mybir.dt.np(mybir.dt.float8e4)

    x = np.ascontiguousarray(x, dtype=np.float32)
    w1 = np.ascontiguousarray(w1, dtype=np.float32)
    w_rec = np.ascontiguousarray(w_rec, dtype=np.float32)
    w2 = np.ascontiguousarray(w2, dtype=np.float32)

    w1t = w1.T                                           # [C, H] f32
    w116 = w1t.astype(np.float16)
    wl = w1t - w116.astype(np.float32)
    w1m = (w116.astype(np.float32) * (-SCALE)).astype(np.float16)
    # fp8 correction factors: products = -S * 2^19 * (w1h*xl + wl*xh)
    w8c = np.clip(-SCALE * (2.0 ** SH_W) * w116.astype(np.float32),
                  -240, 240).astype(f8np)
    wl8 = np.clip(-SCALE * (2.0 ** SH_WL) * wl, -240, 240).astype(f8np)
    wdr = np.stack([w8c, wl8], axis=1)                   # [C, 2, H]

    wrt = (w_rec.T * SCALE).astype(np.float16)           # [H, H]
    w2t = np.ascontiguousarray(w2.T).astype(np.float16)  # [H, O]

    in_maps = []
    for c in range(NCORES):
        xc = x[c * BL:(c + 1) * BL]                      # [BL, T, C]
        xt = np.ascontiguousarray(xc.transpose(2, 1, 0).reshape(C, BT))
        xh = xt.astype(np.float16)
        xl = xt - xh.astype(np.float32)
        xl8 = np.clip(xl * (2.0 ** SH_XL), -240, 240).astype(f8np)
        x8 = np.clip(xh.astype(np.float32) * (2.0 ** SH_X),
                     -240, 240).astype(f8np)
        xdr = np.stack([xl8, x8], axis=1)                # [C, 2, BT]
        in_maps.append({"xh": xh, "xdr": xdr, "w1m": w1m, "wdr": wdr,
                        "wrt": wrt, "w2t": w2t})
    return in_maps


_LAST = {"exec_time_ns": None, "results": None}


def _setup_trace():
    """Register the axon NTFF profiling hook (works without antenv.axon_hooks
    in the image). Only used when SNN_TRACE=1; safe no-op on failure."""
    try:
        import sys
        import types

        import antenv
        if not hasattr(antenv, "axon_hooks"):
            mod = types.ModuleType("antenv.axon_hooks")
            mod._hook = None
            mod.set_axon_ntff_profile_hook = \
                lambda h: setattr(mod, "_hook", h)
            mod.get_axon_ntff_profile_hook = lambda: mod._hook
            sys.modules["antenv.axon_hooks"] = mod
            antenv.axon_hooks = mod
        if antenv.axon_hooks.get_axon_ntff_profile_hook() is None:
            from trn_agent_boot.trn_boot import _ntff_profile_via_ctypes
            hook = _ntff_profile_via_ctypes('/opt/axon/libaxon_pjrt.so')
            if hook is None:
                return False
            antenv.axon_hooks.set_axon_ntff_profile_hook(hook)
        from concourse import bass_utils
        bass_utils.upload_artifacts = lambda tmpdir: tmpdir
        return True
    except Exception:
        return False


def kernel(x, w1, w_rec, w2):
    from concourse.bass_utils import run_bass_kernel_spmd

    nc = build_nc()
    in_maps = prep_inputs(x, w1, w_rec, w2)
    trace = os.environ.get("SNN_TRACE", "0") == "1" and _setup_trace()
    res = run_bass_kernel_spmd(nc, in_maps, list(range(NCORES)), trace=trace)
    _LAST["exec_time_ns"] = res.exec_time_ns
    _LAST["results"] = res
    out = np.empty((B, O), dtype=np.float32)
    for c in range(NCORES):
        out[c * BL:(c + 1) * BL, :] = res.results[c]["out"].T
    return out

